# revision 25
# baseline (speedup 1.0000x reference)
"""Multi-head causal attention with RoPE on 8 TRN2 NeuronCores.

Problem: B=2, T=2048, D=1024, H=16 heads (dh=64), fp32 I/O.
  q/k/v = x @ w{q,k,v}.T ; RoPE(q,k) ; causal softmax((q k^T)/sqrt(dh)) @ v ;
  out = concat_heads @ wo.T

Sharding (8 cores): head-parallel compute, token-striped output. Core c owns
heads {2c, 2c+1} for both batches; four AllToAll collectives redistribute
attention outputs so core c ends up with all 1024 features for its four
128-token chunks {c, c+8, c+16, c+24}; it then applies the full output
projection for those chunks. The host interleaves the chunks back.

v2 scheduling notes (the PE p-state is the whole game: the tensor engine
ramps 0.65 -> 1.2 -> 2.4 GHz only under sustained ~99%+ duty, and drops back
on every stall):
 - All heavy inputs are host-packed into the exact SBUF layout so each DMA is
   one descriptor with 2KB+ contiguous bytes per partition (the v1 layouts
   moved 256B segments at ~22GB/s and stalled the first projections ~17us).
 - PE work is emitted through a filler queue: projection / transpose / final
   matmuls are pulled one instruction at a time between attention QK/PV ops,
   so the in-order PE queue never sits on a long-latency dependency (exp).
 - PV trails QK by two chunks (pt bufs=4) so the scalar-engine exp latency
   (~1.1us) plus semaphore hops never block the PE.
 - A dummy 8-rank AllToAll is dispatched first thing: the collective
   subsystem's bootstrap (~80us, serial on the CC queue) runs concurrently
   with the projection phase instead of delaying a2a(0).
 - a2a_out -> SBUF loads are NOT on the scalar queue (v1 put them ahead of
   the attention exps, which stalled the PE 46us behind a collective), and
   are split per k-chunk across the sync+scalar queues only at points where
   the collective has had a full attention quarter to complete.
 - Engine load map: Scalar = exp only (+tail fo copies); DVE = rope chains,
   o65 psum copies, onr muls, fast reciprocal; Pool = vtt copies, v1
   transposecopies, causal mask muls, denominator broadcasts.
"""

import numpy as np
import ml_dtypes

import concourse.bacc as bacc
import concourse.tile as tile
import concourse.mybir as mybir
from concourse import bass_utils

BF16 = mybir.dt.bfloat16
F32 = mybir.dt.float32
AF = mybir.ActivationFunctionType

NCORES = 8
B, T, D, H = 2, 2048, 1024, 16
DH = D // H          # 64
HPC = H // NCORES    # 2 heads per core
FPC = DH * HPC       # 128 features per core
TOK = B * T          # 4096
TPC = TOK // NCORES  # 512 tokens per core (output shard)
KC = D // 128        # 8 contraction chunks
NT = T // 512        # 4 query tiles of 512 per batch
VG = 256             # cols per v-group: [v_h0(64) | 1 | pad | v_h1(64) | 1 | pad]

_COMPILED = None


def _build():
    nc = bacc.Bacc("TRN2", target_bir_lowering=False, debug=False, num_devices=NCORES)

    xp_d = nc.dram_tensor("xp", [128, KC * TOK], BF16, kind="ExternalInput")
    wq_d = nc.dram_tensor("wqp", [128, KC * FPC], BF16, kind="ExternalInput")
    wk_d = nc.dram_tensor("wkp", [128, KC * FPC], BF16, kind="ExternalInput")
    wv_d = nc.dram_tensor("wvp", [128, KC * FPC], BF16, kind="ExternalInput")
    wo_d = nc.dram_tensor("wop", [128, KC * D], BF16, kind="ExternalInput")
    C_d = nc.dram_tensor("cosC", [128, T], BF16, kind="ExternalInput")
    S_d = nc.dram_tensor("sinS", [128, T], BF16, kind="ExternalInput")
    mask_d = nc.dram_tensor("mask", [128, 128], BF16, kind="ExternalInput")
    id_d = nc.dram_tensor("ident", [128, 128], BF16, kind="ExternalInput")
    out_d = nc.dram_tensor("out", [TPC, D], F32, kind="ExternalOutput")

    swap16 = list(range(16, 32)) + list(range(16))

    with tile.TileContext(nc) as tc:
        with (
            tc.tile_pool(name="sb", bufs=1) as sb,
            tc.tile_pool(name="ps", bufs=1, space="PSUM") as ps,
            tc.tile_pool(name="dram", bufs=1, space="DRAM") as dram,
        ):
            # ---- dummy collective first: pays the CC bootstrap cost during
            # the projection phase (the CC queue is serial; in v1 the ~80us
            # bootstrap delayed a2a(0) until ~100us in) ----
            dum_in = dram.tile([8, 16], BF16, name="dumin")
            dum_out = dram.tile([8, 16], BF16, name="dumout")
            zz = sb.tile([8, 16], BF16)
            nc.gpsimd.memset(zz[:], 0.0)
            nc.gpsimd.dma_start(dum_in[:], zz[:])
            nc.gpsimd.collective_compute(
                "AllToAll",
                mybir.AluOpType.bypass,
                replica_groups=[list(range(NCORES))],
                ins=[dum_in.opt()],
                outs=[dum_out.opt()],
            )

            # ---- prefetch: everything is host-packed, one flat DMA each.
            # scalar queue: weights + rope tables + odd x blocks + wo
            # sync queue:   even x blocks + mask + identity
            wq_sb = sb.tile([128, KC * FPC], BF16)
            wk_sb = sb.tile([128, KC * FPC], BF16)
            wv_sb = sb.tile([128, KC * FPC], BF16)
            C_sb = sb.tile([128, T], BF16)
            S_sb = sb.tile([128, T], BF16)
            mask2_sb = sb.tile([128, 256], BF16)
            id_sb = sb.tile([128, 128], BF16)
            xp_sb = sb.tile([128, KC * TOK], BF16)
            wo_sb = sb.tile([128, KC * D], BF16)

            BLK = KC * 512  # 4096 cols per (b,n) token block

            def x_block(i):
                return (
                    xp_sb[:, BLK * i : BLK * i + BLK],
                    xp_d[:, BLK * i : BLK * i + BLK],
                )

            # first token block on sync so the first projection starts ASAP
            d, s = x_block(0)
            nc.sync.dma_start(d, s)
            nc.scalar.dma_start(wq_sb[:], wq_d[:])
            nc.scalar.dma_start(wk_sb[:], wk_d[:])
            nc.scalar.dma_start(wv_sb[:], wv_d[:])
            nc.sync.dma_start(C_sb[:], C_d[:])
            nc.sync.dma_start(S_sb[:], S_d[:])
            nc.sync.dma_start(mask2_sb[:, 0:128], mask_d[:])
            nc.sync.dma_start(mask2_sb[:, 128:256], mask_d[:])
            nc.sync.dma_start(id_sb[:], id_d[:])
            for i in range(1, B * NT):
                d, s = x_block(i)
                (nc.scalar if i % 2 else nc.sync).dma_start(d, s)
            nc.scalar.dma_start(wo_sb[:], wo_d[:])

            # ---- persistent intermediates ----
            qrot_sb = sb.tile([128, TOK], BF16)
            krot_sb = sb.tile([128, TOK], BF16)
            v1_sb = sb.tile([128, B * (T // 128) * VG], BF16)
            nc.gpsimd.memset(
                v1_sb[:].rearrange("p (g c) -> p g c", c=128)[:, :, 64:65], 1.0
            )

            # 4 AllToAll groups: group g carries global token chunks 8g+o to rank o
            a2a_in = [dram.tile([D, 128], BF16, name=f"a2ain{g}") for g in range(4)]
            a2a_out = [dram.tile([D, 128], BF16, name=f"a2aout{g}") for g in range(4)]

            # ================= filler machinery =================
            fillers = []  # FIFO of zero-arg thunks, each ~1 PE instruction

            def pull(k):
                for _ in range(k):
                    if fillers:
                        fillers.pop(0)()

            def drain():
                while fillers:
                    fillers.pop(0)()

            def rope_tile(pp, dst_sb, b, n):
                # dst = pp*C + swap16(pp)*S, reading the projection psum directly
                swp = sb.tile([128, 512], F32, tag="swp", bufs=3, name=f"swp{b}{n}")
                nc.vector.stream_shuffle(swp[:], pp[:], swap16)
                t1 = sb.tile([128, 512], BF16, tag="t1", bufs=3, name=f"t1{b}{n}")
                nc.vector.tensor_mul(t1[:], pp[:], C_sb[:, 512 * n : 512 * n + 512])
                # SBUF-only operands -> Pool engine (PSUM is DVE/Act-only)
                t2 = sb.tile([128, 512], BF16, tag="t2", bufs=3, name=f"t2{b}{n}")
                nc.gpsimd.tensor_mul(t2[:], swp[:], S_sb[:, 512 * n : 512 * n + 512])
                nc.vector.tensor_add(
                    dst_sb[:, b * T + 512 * n : b * T + 512 * n + 512], t1[:], t2[:]
                )

            def add_proj_fillers(w_sb, b, n, kind):
                """8 fillers (one matmul each). kind: 'q'|'k'|'v'. The last
                filler emits the rope (DVE) or the vtt copy (Pool) + queues
                the 4 transpose fillers (they land at the back of the queue,
                by which time vtt is long done)."""
                st = {}
                blk = (NT * b + n) * BLK

                def mk(kc):
                    def f():
                        if kc == 0:
                            st["pp"] = ps.tile(
                                [128, 512], F32, tag="proj", bufs=2, name=f"pp{kind}{b}{n}"
                            )
                        nc.tensor.matmul(
                            st["pp"][:],
                            w_sb[:, kc * FPC : (kc + 1) * FPC],
                            xp_sb[:, blk + 512 * kc : blk + 512 * kc + 512],
                            start=(kc == 0),
                            stop=(kc == KC - 1),
                        )
                        if kc == KC - 1:
                            if kind == "q":
                                rope_tile(st["pp"], qrot_sb, b, n)
                            elif kind == "k":
                                rope_tile(st["pp"], krot_sb, b, n)
                            else:
                                vtt = sb.tile(
                                    [128, 512], BF16, tag="vtt", bufs=2, name=f"vtt{b}{n}"
                                )
                                nc.scalar.activation(vtt[:], st["pp"][:], AF.Copy)
                                for i in range(4):
                                    fillers.append(mk_transpose(vtt, b, n, i))

                    return f

                for kc in range(KC):
                    fillers.append(mk(kc))

            def mk_transpose(vtt, b, n, i):
                def f():
                    g = VG * ((T // 128) * b + 4 * n + i)
                    tp = ps.tile([128, 128], BF16, tag="proj", bufs=2, name=f"tp{b}{n}{i}")
                    nc.tensor.matmul(
                        tp[:],
                        vtt[:, 128 * i : 128 * i + 128],
                        id_sb[:],
                        is_transpose=True,
                        start=True,
                        stop=True,
                    )
                    # one strided DVE copy moves both heads' 64 columns
                    # (PSUM source: DVE/Act only)
                    nc.vector.tensor_copy(
                        v1_sb[:, g : g + 256].rearrange("p (h c) -> p h c", h=2)[
                            :, :, 0:64
                        ],
                        tp[:].rearrange("p (h c) -> p h c", h=2),
                    )

                return f

            def add_tile(b, n):
                add_proj_fillers(wq_sb, b, n, "q")
                add_proj_fillers(wk_sb, b, n, "k")
                add_proj_fillers(wv_sb, b, n, "v")

            # ---- at tiles (a2a_out -> SBUF), split per k-chunk ----
            at_tiles = {}

            def load_at(g, queues):
                at = sb.tile([128, KC * 128], BF16, tag="at", bufs=4, name=f"at{g}")
                for kc in range(KC):
                    q = queues[kc % len(queues)]
                    q.dma_start(
                        at[:, 128 * kc : 128 * kc + 128],
                        a2a_out[g][128 * kc : 128 * kc + 128, :],
                    )
                at_tiles[g] = at

            def add_final_fillers(g):
                """16 fillers: output projection for token chunk group g."""
                st = {}

                def mk(nh, kc):
                    def f():
                        at = at_tiles[g]
                        if kc == 0:
                            st[nh] = ps.tile(
                                [128, 512], F32, tag="proj", bufs=2, name=f"fp{g}{nh}"
                            )
                        nc.tensor.matmul(
                            st[nh][:],
                            at[:, 128 * kc : 128 * kc + 128],
                            wo_sb[:, kc * D + 512 * nh : kc * D + 512 * nh + 512],
                            start=(kc == 0),
                            stop=(kc == KC - 1),
                        )
                        if kc == KC - 1:
                            # DVE copy: the scalar queue stays exp-only so
                            # finals pulled mid-attention never delay an exp
                            fo = sb.tile(
                                [128, 512], F32, tag="fo", bufs=2, name=f"fo{g}{nh}"
                            )
                            nc.vector.tensor_copy(fo[:], st[nh][:])
                            nc.sync.dma_start(
                                out_d[128 * g : 128 * g + 128, 512 * nh : 512 * nh + 512],
                                fo[:],
                            )

                    return f

                for nh in range(2):
                    for kc in range(KC):
                        fillers.append(mk(nh, kc))

            # ================= attention =================
            def attn_core(b, j, pending_cc=None, filler_delay=0):
                """Both heads for (batch b, q-tile j). QK pairs run in disjoint
                PE row groups; exp on the scalar engine; causal mask mul on
                Pool; PV trails QK by TWO chunks so exp latency never stalls
                the in-order PE queue. Fillers are pulled between chunk ops.
                Returns the two [65,512] f32 SBUF copies of the O accumulators
                (row 64 = softmax sums)."""
                ops = [
                    ps.tile([65, 512], F32, tag="opsum", bufs=2, name=f"op{b}{h}{j}")
                    for h in range(2)
                ]
                nch = 4 * j + 4

                def qk_exp(c):
                    diag = c - 4 * j
                    lo = 128 * diag if diag >= 0 else 0
                    sp = ps.tile(
                        [128, 1024], F32, tag="spsum", bufs=2, name=f"sp{b}{j}{c}"
                    )
                    spv = sp[:].rearrange("p (h t) -> p h t", h=2)
                    for h in range(2):
                        nc.tensor.matmul(
                            sp[:, 512 * h + lo : 512 * h + 512],
                            krot_sb[64 * h : 64 * h + 64, b * T + 128 * c : b * T + 128 * c + 128],
                            qrot_sb[
                                64 * h : 64 * h + 64,
                                b * T + 512 * j + lo : b * T + 512 * j + 512,
                            ],
                            start=True,
                            stop=True,
                        )
                    pt = sb.tile(
                        [128, 1024], BF16, tag="pt", bufs=4, name=f"pt{b}{j}{c}"
                    )
                    ptv = pt[:].rearrange("p (h t) -> p h t", h=2)
                    nc.scalar.activation(
                        ptv[:, :, lo:512], spv[:, :, lo:512], AF.Exp, scale=0.125
                    )
                    if diag >= 0:
                        # zero the upper triangle post-exp (Pool engine; DVE is
                        # busy with rope chains pulled as fillers)
                        nc.gpsimd.tensor_mul(
                            ptv[:, :, lo : lo + 128], ptv[:, :, lo : lo + 128],
                            mask2_sb[:].rearrange("p (h t) -> p h t", h=2),
                        )
                    return pt

                def pv(c, pt):
                    diag = c - 4 * j
                    lo = 128 * diag if diag >= 0 else 0
                    g = VG * ((T // 128) * b + c)
                    for h in range(2):
                        nc.tensor.matmul(
                            ops[h][:, lo:512],
                            v1_sb[:, g + 128 * h : g + 128 * h + 65],
                            pt[:, 512 * h + lo : 512 * h + 512],
                            start=(c == 0),
                            stop=(c == nch - 1),
                        )

                pts = {}
                for c in range(nch):
                    pts[c] = qk_exp(c)
                    if c >= filler_delay:
                        pull(2)
                    if c == 3 and pending_cc is not None:
                        # late dispatch: the a2a_in staging DMAs have had ~3
                        # chunk-steps to land, so the Pool queue (which hosts
                        # the collective dispatch) barely waits
                        pending_cc()
                    if c >= 2:
                        pv(c - 2, pts.pop(c - 2))
                        if c >= filler_delay:
                            pull(1)
                pv(nch - 2, pts.pop(nch - 2))
                pull(2)
                pv(nch - 1, pts.pop(nch - 1))
                # psum -> SBUF copies on DVE release the opsum banks for the
                # next tile without loading the exp-critical scalar queue
                o65s = []
                for h in range(2):
                    o65 = sb.tile([65, 512], F32, tag="o65", bufs=4, name=f"o65{b}{h}{j}")
                    nc.vector.tensor_copy(o65[:], ops[h][:])
                    o65s.append(o65)
                return o65s

            def attn_epilogue(b, j, o65s):
                """Normalize one q-tile's outputs (2 head-tiles): fast
                reciprocal, Pool partition-broadcast instead of the v1
                sel-matmul (saves PE rows), then one merged staging DMA per
                head into the a2a buffer. Per-j epilogues keep the staging
                bursts small and start each collective's inputs a full tile
                earlier than the v1 pair-epilogue."""
                sums = sb.tile([2, 512], F32, tag="sums", bufs=3, name=f"sums{b}{j}")
                for h in range(2):
                    nc.sync.dma_start(sums[h : h + 1, :], o65s[h][64:65, :])
                rec2 = sb.tile([2, 512], F32, tag="rec4", bufs=3, name=f"rec2{b}{j}")
                nc.vector.reciprocal_approx_fast(rec2[:], sums[:])
                # partition_broadcast sources must start at partition 0:
                # gather the reciprocal rows into one partition first
                rb1 = sb.tile([1, 2 * 512], F32, tag="rb1", bufs=3, name=f"rb1{b}{j}")
                for h in range(2):
                    nc.sync.dma_start(
                        rb1[0:1, 512 * h : 512 * h + 512], rec2[h : h + 1, :]
                    )
                m0 = 16 * b + 4 * j
                o0, g = m0 % 8, m0 // 8
                for h in range(2):
                    rec64 = sb.tile([64, 512], F32, tag="rec64", bufs=4, name=f"rec64{b}{j}{h}")
                    nc.gpsimd.partition_broadcast(rec64[:], rb1[0:1, 512 * h : 512 * h + 512])
                    onr = sb.tile([64, 512], BF16, tag="onr", bufs=4, name=f"onr{b}{j}{h}")
                    nc.vector.tensor_mul(onr[:], o65s[h][0:64, :], rec64[:])
                    # all 4 chunks in one DMA: dst rows 128*(o0+i)+64h..+64
                    nc.sync.dma_start(
                        a2a_in[g][:]
                        .rearrange("(o r) t -> r o t", r=128)[
                            64 * h : 64 * h + 64, o0 : o0 + 4, :
                        ],
                        onr[:].rearrange("p (i t) -> p i t", i=4),
                    )

            def a2a_call(g):
                def f():
                    nc.gpsimd.collective_compute(
                        "AllToAll",
                        mybir.AluOpType.bypass,
                        replica_groups=[list(range(NCORES))],
                        ins=[a2a_in[g].opt()],
                        outs=[a2a_out[g].opt()],
                    )

                return f

            # ================= schedule =================
            # prolog: three tiles emitted sequentially (attention can't start
            # before (0,0) anyway; (0,2) keeps the filler source 2 tiles ahead)
            add_tile(0, 0)
            drain()
            add_tile(0, 1)
            drain()
            add_tile(0, 2)
            drain()

            add_tile(0, 3)
            attn_epilogue(0, 0, attn_core(0, 0))
            add_tile(1, 0)
            attn_epilogue(0, 1, attn_core(0, 1))  # cc0 inputs complete here
            add_tile(1, 1)
            attn_epilogue(0, 2, attn_core(0, 2, pending_cc=a2a_call(0)))
            add_tile(1, 2)
            add_tile(1, 3)
            attn_epilogue(0, 3, attn_core(0, 3))  # cc1 inputs complete here
            # cc0 finished during attn(0,3): load its output during attn(1,0)
            load_at(0, [nc.sync, nc.scalar])
            attn_epilogue(1, 0, attn_core(1, 0, pending_cc=a2a_call(1)))
            add_final_fillers(0)
            # cc2 inputs complete after attn(1,1); small filler delay gives
            # the at(0) chunk loads a head start over the final(0) pulls
            attn_epilogue(1, 1, attn_core(1, 1, filler_delay=2))
            load_at(1, [nc.sync, nc.scalar])
            add_final_fillers(1)
            # final(1) pulled only from chunk 4 on: cc1 lands mid-attn(1,2),
            # so early pulls would stall the PE on the at(1) load
            attn_epilogue(
                1, 2, attn_core(1, 2, pending_cc=a2a_call(2), filler_delay=4)
            )
            # no fillers for attn(1,3): final(2) is reserved to overlap cc3
            attn_epilogue(1, 3, attn_core(1, 3))  # cc3 inputs complete here
            a2a_call(3)()
            load_at(2, [nc.sync, nc.scalar])
            add_final_fillers(2)
            drain()  # final(2) runs back-to-back while cc3 flies
            load_at(3, [nc.sync, nc.scalar, nc.gpsimd])
            add_final_fillers(3)
            drain()

    nc.compile()
    return nc


def _get_compiled():
    global _COMPILED
    if _COMPILED is None:
        _COMPILED = _build()
    return _COMPILED


def _prep_in_maps(embedding_word, wq, wk, wv, wo):
    bf = ml_dtypes.bfloat16
    x = np.asarray(embedding_word, np.float32).reshape(TOK, D)
    xT = np.ascontiguousarray(x.T).astype(bf)  # [D, TOK]
    # pack to [128, (b n k t)]: token block (b,n) = 4096 contiguous cols,
    # k-chunk major inside, so every DMA row segment is contiguous
    xp = np.ascontiguousarray(
        xT.reshape(KC, 128, B, NT, 512).transpose(1, 2, 3, 0, 4).reshape(128, KC * TOK)
    )

    woT = np.asarray(wo, np.float32).T  # [D, D]
    wop = np.ascontiguousarray(
        woT.reshape(KC, 128, D).transpose(1, 0, 2).reshape(128, KC * D)
    ).astype(bf)

    # within-head row permutation: 16 re rows then 16 im rows per 32-row quadrant
    perm64 = [
        (2 * (16 * q + r) if r < 16 else 2 * (16 * q + (r - 16)) + 1)
        for q in range(2)
        for r in range(32)
    ]
    perm64 = np.asarray(perm64)

    freqs = 1.0 / (10000.0 ** (np.arange(0, DH, 2, dtype=np.float64) / DH))  # [32]
    ang = np.arange(T, dtype=np.float64)[:, None] * freqs[None, :]  # [T, 32]
    cos_t, sin_t = np.cos(ang), np.sin(ang)
    rows = np.arange(128)
    wh = rows % 64
    qd = wh // 32
    r32 = wh % 32
    dmap = 16 * qd + (r32 % 16)
    sign = np.where(r32 < 16, -1.0, 1.0)
    C = np.ascontiguousarray(cos_t[:, dmap].T).astype(bf)  # [128, T]
    S = np.ascontiguousarray((sin_t[:, dmap] * sign[None, :]).T).astype(bf)

    rr = np.arange(128)[:, None]
    cc = np.arange(128)[None, :]
    mask = np.where(cc >= rr, 1.0, 0.0).astype(bf)
    ident = np.eye(128, dtype=np.float32).astype(bf)

    wqf = np.asarray(wq, np.float32)
    wkf = np.asarray(wk, np.float32)
    wvf = np.asarray(wv, np.float32)

    def pack_w(w_c):
        # w_c: [FPC, D] -> transpose -> [D, FPC] -> [128, (k c)] SBUF layout
        wT = w_c.T
        return np.ascontiguousarray(
            wT.reshape(KC, 128, FPC).transpose(1, 0, 2).reshape(128, KC * FPC)
        ).astype(bf)

    in_maps = []
    for c in range(NCORES):
        rows_c = slice(FPC * c, FPC * c + FPC)
        wq_c = wqf[rows_c].reshape(HPC, DH, D)[:, perm64, :].reshape(FPC, D)
        wk_c = wkf[rows_c].reshape(HPC, DH, D)[:, perm64, :].reshape(FPC, D)
        wv_c = wvf[rows_c]
        in_maps.append(
            {
                "xp": xp,
                "wqp": pack_w(wq_c),
                "wkp": pack_w(wk_c),
                "wvp": pack_w(wv_c),
                "wop": wop,
                "cosC": C,
                "sinS": S,
                "mask": mask,
                "ident": ident,
            }
        )
    return in_maps


def _unshard(core_outs):
    """core_outs[c] is [TPC, D] covering token chunks {c, 8+c, 16+c, 24+c}
    (row-blocks g=0..3). Interleave back to [B, T, D]."""
    a = np.stack(core_outs, axis=0)  # [8, TPC, D]
    a = a.reshape(NCORES, 4, 128, D).transpose(1, 0, 2, 3).reshape(TOK, D)
    return np.ascontiguousarray(a.reshape(B, T, D).astype(np.float32))


def kernel(embedding_word, wq, wk, wv, wo):
    nc = _get_compiled()
    in_maps = _prep_in_maps(embedding_word, wq, wk, wv, wo)
    res = bass_utils.run_bass_kernel_spmd(nc, in_maps, core_ids=list(range(NCORES)))
    return _unshard([res.results[c]["out"] for c in range(NCORES)])


# revision 33
# speedup vs baseline: 1.0162x; 1.0162x over previous
"""Multi-head causal attention with RoPE on 8 TRN2 NeuronCores.

Problem: B=2, T=2048, D=1024, H=16 heads (dh=64), fp32 I/O.
  q/k/v = x @ w{q,k,v}.T ; RoPE(q,k) ; causal softmax((q k^T)/sqrt(dh)) @ v ;
  out = concat_heads @ wo.T

Sharding (8 cores): head-parallel compute, token-striped output. Core c owns
heads {2c, 2c+1} for both batches; four AllToAll collectives redistribute
attention outputs so core c ends up with all 1024 features for its four
128-token chunks {c, c+8, c+16, c+24}; it then applies the full output
projection for those chunks. The host interleaves the chunks back.

v2 scheduling notes (the PE p-state is the whole game: the tensor engine
ramps 0.65 -> 1.2 -> 2.4 GHz only under sustained ~99%+ duty, and drops back
on every stall):
 - All heavy inputs are host-packed into the exact SBUF layout so each DMA is
   one descriptor with 2KB+ contiguous bytes per partition (the v1 layouts
   moved 256B segments at ~22GB/s and stalled the first projections ~17us).
 - PE work is emitted through a filler queue: projection / transpose / final
   matmuls are pulled one instruction at a time between attention QK/PV ops,
   so the in-order PE queue never sits on a long-latency dependency (exp).
 - PV trails QK by two chunks (pt bufs=4) so the scalar-engine exp latency
   (~1.1us) plus semaphore hops never block the PE.
 - A dummy 8-rank AllToAll is dispatched first thing: the collective
   subsystem's bootstrap (~80us, serial on the CC queue) runs concurrently
   with the projection phase instead of delaying a2a(0).
 - a2a_out -> SBUF loads are NOT on the scalar queue (v1 put them ahead of
   the attention exps, which stalled the PE 46us behind a collective), and
   are split per k-chunk across the sync+scalar queues only at points where
   the collective has had a full attention quarter to complete.
 - Engine load map: Scalar = exp only (+tail fo copies); DVE = rope chains,
   o65 psum copies, onr muls, fast reciprocal; Pool = vtt copies, v1
   transposecopies, causal mask muls, denominator broadcasts.
"""

import numpy as np
import ml_dtypes

import concourse.bacc as bacc
import concourse.tile as tile
import concourse.mybir as mybir
from concourse import bass_utils

BF16 = mybir.dt.bfloat16
F32 = mybir.dt.float32
AF = mybir.ActivationFunctionType

NCORES = 8
B, T, D, H = 2, 2048, 1024, 16
DH = D // H          # 64
HPC = H // NCORES    # 2 heads per core
FPC = DH * HPC       # 128 features per core
TOK = B * T          # 4096
TPC = TOK // NCORES  # 512 tokens per core (output shard)
KC = D // 128        # 8 contraction chunks
NT = T // 512        # 4 query tiles of 512 per batch
VG = 256             # cols per v-group: [v_h0(64) | 1 | pad | v_h1(64) | 1 | pad]

_COMPILED = None


def _build():
    nc = bacc.Bacc("TRN2", target_bir_lowering=False, debug=False, num_devices=NCORES)

    xp_d = nc.dram_tensor("xp", [128, KC * TOK], BF16, kind="ExternalInput")
    wq_d = nc.dram_tensor("wqp", [128, KC * FPC], BF16, kind="ExternalInput")
    wk_d = nc.dram_tensor("wkp", [128, KC * FPC], BF16, kind="ExternalInput")
    wv_d = nc.dram_tensor("wvp", [128, KC * FPC], BF16, kind="ExternalInput")
    wo_d = nc.dram_tensor("wop", [128, KC * D], BF16, kind="ExternalInput")
    C_d = nc.dram_tensor("cosC", [128, T], BF16, kind="ExternalInput")
    S_d = nc.dram_tensor("sinS", [128, T], BF16, kind="ExternalInput")
    mask_d = nc.dram_tensor("mask", [128, 128], BF16, kind="ExternalInput")
    id_d = nc.dram_tensor("ident", [128, 128], BF16, kind="ExternalInput")
    out_d = nc.dram_tensor("out", [TPC, D], F32, kind="ExternalOutput")

    swap16 = list(range(16, 32)) + list(range(16))

    with tile.TileContext(nc) as tc:
        with (
            tc.tile_pool(name="sb", bufs=1) as sb,
            tc.tile_pool(name="ps", bufs=1, space="PSUM") as ps,
            tc.tile_pool(name="dram", bufs=1, space="DRAM") as dram,
        ):
            # ---- dummy collective first: pays the CC bootstrap cost during
            # the projection phase (the CC queue is serial; in v1 the ~80us
            # bootstrap delayed a2a(0) until ~100us in) ----
            dum_in = dram.tile([8, 16], BF16, name="dumin")
            dum_out = dram.tile([8, 16], BF16, name="dumout")
            zz = sb.tile([8, 16], BF16)
            nc.gpsimd.memset(zz[:], 0.0)
            nc.gpsimd.dma_start(dum_in[:], zz[:])
            nc.gpsimd.collective_compute(
                "AllToAll",
                mybir.AluOpType.bypass,
                replica_groups=[list(range(NCORES))],
                ins=[dum_in.opt()],
                outs=[dum_out.opt()],
            )

            # ---- prefetch: everything is host-packed, one flat DMA each.
            # scalar queue: weights + rope tables + odd x blocks + wo
            # sync queue:   even x blocks + mask + identity
            wq_sb = sb.tile([128, KC * FPC], BF16)
            wk_sb = sb.tile([128, KC * FPC], BF16)
            wv_sb = sb.tile([128, KC * FPC], BF16)
            C_sb = sb.tile([128, T], BF16)
            S_sb = sb.tile([128, T], BF16)
            mask2_sb = sb.tile([128, 256], BF16)
            id_sb = sb.tile([128, 128], BF16)
            xp_sb = sb.tile([128, KC * TOK], BF16)
            wo_sb = sb.tile([128, KC * D], BF16)

            BLK = KC * 512  # 4096 cols per (b,n) token block

            def x_block(i):
                return (
                    xp_sb[:, BLK * i : BLK * i + BLK],
                    xp_d[:, BLK * i : BLK * i + BLK],
                )

            # first token block on sync so the first projection starts ASAP
            d, s = x_block(0)
            nc.sync.dma_start(d, s)
            nc.scalar.dma_start(wq_sb[:], wq_d[:])
            nc.scalar.dma_start(wk_sb[:], wk_d[:])
            nc.scalar.dma_start(wv_sb[:], wv_d[:])
            nc.sync.dma_start(C_sb[:], C_d[:])
            nc.sync.dma_start(S_sb[:], S_d[:])
            nc.sync.dma_start(mask2_sb[:, 0:128], mask_d[:])
            nc.sync.dma_start(mask2_sb[:, 128:256], mask_d[:])
            nc.sync.dma_start(id_sb[:], id_d[:])
            for i in range(1, B * NT):
                d, s = x_block(i)
                (nc.scalar if i % 2 else nc.sync).dma_start(d, s)
            nc.scalar.dma_start(wo_sb[:], wo_d[:])

            # ---- persistent intermediates ----
            qrot_sb = sb.tile([128, TOK], BF16)
            krot_sb = sb.tile([128, TOK], BF16)
            v1_sb = sb.tile([128, B * (T // 128) * VG], BF16)
            nc.gpsimd.memset(
                v1_sb[:].rearrange("p (g c) -> p g c", c=128)[:, :, 64:65], 1.0
            )

            # 4 AllToAll groups: group g carries global token chunks 8g+o to rank o
            a2a_in = [dram.tile([D, 128], BF16, name=f"a2ain{g}") for g in range(4)]
            a2a_out = [dram.tile([D, 128], BF16, name=f"a2aout{g}") for g in range(4)]

            # ================= filler machinery =================
            fillers = []  # FIFO of zero-arg thunks, each ~1 PE instruction

            def pull(k):
                for _ in range(k):
                    if fillers:
                        fillers.pop(0)()

            def drain():
                while fillers:
                    fillers.pop(0)()

            def rope_tile(pp, dst_sb, b, n):
                # dst = pp*C + swap16(pp)*S, reading the projection psum directly
                swp = sb.tile([128, 512], F32, tag="swp", bufs=3, name=f"swp{b}{n}")
                nc.vector.stream_shuffle(swp[:], pp[:], swap16)
                t1 = sb.tile([128, 512], BF16, tag="t1", bufs=3, name=f"t1{b}{n}")
                nc.vector.tensor_mul(t1[:], pp[:], C_sb[:, 512 * n : 512 * n + 512])
                t2 = sb.tile([128, 512], BF16, tag="t2", bufs=3, name=f"t2{b}{n}")
                nc.vector.tensor_mul(t2[:], swp[:], S_sb[:, 512 * n : 512 * n + 512])
                nc.vector.tensor_add(
                    dst_sb[:, b * T + 512 * n : b * T + 512 * n + 512], t1[:], t2[:]
                )

            def add_proj_fillers(w_sb, b, n, kind):
                """8 fillers (one matmul each). kind: 'q'|'k'|'v'. The last
                filler emits the rope (DVE) or the vtt copy (Pool) + queues
                the 4 transpose fillers (they land at the back of the queue,
                by which time vtt is long done)."""
                st = {}
                blk = (NT * b + n) * BLK

                def mk(kc):
                    def f():
                        if kc == 0:
                            st["pp"] = ps.tile(
                                [128, 512], F32, tag="proj", bufs=2, name=f"pp{kind}{b}{n}"
                            )
                        nc.tensor.matmul(
                            st["pp"][:],
                            w_sb[:, kc * FPC : (kc + 1) * FPC],
                            xp_sb[:, blk + 512 * kc : blk + 512 * kc + 512],
                            start=(kc == 0),
                            stop=(kc == KC - 1),
                        )
                        if kc == KC - 1:
                            if kind == "q":
                                rope_tile(st["pp"], qrot_sb, b, n)
                            elif kind == "k":
                                rope_tile(st["pp"], krot_sb, b, n)
                            else:
                                vtt = sb.tile(
                                    [128, 512], BF16, tag="vtt", bufs=2, name=f"vtt{b}{n}"
                                )
                                nc.scalar.activation(vtt[:], st["pp"][:], AF.Copy)
                                for i in range(4):
                                    fillers.append(mk_transpose(vtt, b, n, i))

                    return f

                for kc in range(KC):
                    fillers.append(mk(kc))

            def mk_transpose(vtt, b, n, i):
                def f():
                    g = VG * ((T // 128) * b + 4 * n + i)
                    tp = ps.tile([128, 128], BF16, tag="proj", bufs=2, name=f"tp{b}{n}{i}")
                    nc.tensor.matmul(
                        tp[:],
                        vtt[:, 128 * i : 128 * i + 128],
                        id_sb[:],
                        is_transpose=True,
                        start=True,
                        stop=True,
                    )
                    # one strided DVE copy moves both heads' 64 columns
                    # (PSUM source: DVE/Act only)
                    nc.vector.tensor_copy(
                        v1_sb[:, g : g + 256].rearrange("p (h c) -> p h c", h=2)[
                            :, :, 0:64
                        ],
                        tp[:].rearrange("p (h c) -> p h c", h=2),
                    )

                return f

            def add_tile(b, n):
                add_proj_fillers(wq_sb, b, n, "q")
                add_proj_fillers(wk_sb, b, n, "k")
                add_proj_fillers(wv_sb, b, n, "v")

            # ---- at tiles (a2a_out -> SBUF), split per k-chunk ----
            at_tiles = {}

            def load_at(g, queues):
                at = sb.tile([128, KC * 128], BF16, tag="at", bufs=4, name=f"at{g}")
                for kc in range(KC):
                    q = queues[kc % len(queues)]
                    q.dma_start(
                        at[:, 128 * kc : 128 * kc + 128],
                        a2a_out[g][128 * kc : 128 * kc + 128, :],
                    )
                at_tiles[g] = at

            def add_final_fillers(g):
                """16 fillers: output projection for token chunk group g."""
                st = {}

                def mk(nh, kc):
                    def f():
                        at = at_tiles[g]
                        if kc == 0:
                            st[nh] = ps.tile(
                                [128, 512], F32, tag="proj", bufs=2, name=f"fp{g}{nh}"
                            )
                        nc.tensor.matmul(
                            st[nh][:],
                            at[:, 128 * kc : 128 * kc + 128],
                            wo_sb[:, kc * D + 512 * nh : kc * D + 512 * nh + 512],
                            start=(kc == 0),
                            stop=(kc == KC - 1),
                        )
                        if kc == KC - 1:
                            # DVE copy: the scalar queue stays exp-only so
                            # finals pulled mid-attention never delay an exp
                            fo = sb.tile(
                                [128, 512], F32, tag="fo", bufs=2, name=f"fo{g}{nh}"
                            )
                            nc.vector.tensor_copy(fo[:], st[nh][:])
                            nc.sync.dma_start(
                                out_d[128 * g : 128 * g + 128, 512 * nh : 512 * nh + 512],
                                fo[:],
                            )

                    return f

                for nh in range(2):
                    for kc in range(KC):
                        fillers.append(mk(nh, kc))

            # ================= attention =================
            def attn_core(b, j, hooks=None, filler_delay=0):
                """Both heads for (batch b, q-tile j). QK pairs run in disjoint
                PE row groups; exp on the scalar engine; causal mask mul on
                Pool; PV trails QK by TWO chunks so exp latency never stalls
                the in-order PE queue. Fillers are pulled between chunk ops.
                Returns the two [65,512] f32 SBUF copies of the O accumulators
                (row 64 = softmax sums)."""
                ops = [
                    ps.tile([65, 512], F32, tag="opsum", bufs=2, name=f"op{b}{h}{j}")
                    for h in range(2)
                ]
                nch = 4 * j + 4

                def qk_exp(c):
                    diag = c - 4 * j
                    lo = 128 * diag if diag >= 0 else 0
                    sp = ps.tile(
                        [128, 1024], F32, tag="spsum", bufs=2, name=f"sp{b}{j}{c}"
                    )
                    spv = sp[:].rearrange("p (h t) -> p h t", h=2)
                    for h in range(2):
                        nc.tensor.matmul(
                            sp[:, 512 * h + lo : 512 * h + 512],
                            krot_sb[64 * h : 64 * h + 64, b * T + 128 * c : b * T + 128 * c + 128],
                            qrot_sb[
                                64 * h : 64 * h + 64,
                                b * T + 512 * j + lo : b * T + 512 * j + 512,
                            ],
                            start=True,
                            stop=True,
                        )
                    pt = sb.tile(
                        [128, 1024], BF16, tag="pt", bufs=4, name=f"pt{b}{j}{c}"
                    )
                    ptv = pt[:].rearrange("p (h t) -> p h t", h=2)
                    nc.scalar.activation(
                        ptv[:, :, lo:512], spv[:, :, lo:512], AF.Exp, scale=0.125
                    )
                    if diag >= 0:
                        # zero the upper triangle post-exp (Pool engine; DVE is
                        # busy with rope chains pulled as fillers)
                        nc.gpsimd.tensor_mul(
                            ptv[:, :, lo : lo + 128], ptv[:, :, lo : lo + 128],
                            mask2_sb[:].rearrange("p (h t) -> p h t", h=2),
                        )
                    return pt

                def pv(c, pt):
                    diag = c - 4 * j
                    lo = 128 * diag if diag >= 0 else 0
                    g = VG * ((T // 128) * b + c)
                    for h in range(2):
                        nc.tensor.matmul(
                            ops[h][:, lo:512],
                            v1_sb[:, g + 128 * h : g + 128 * h + 65],
                            pt[:, 512 * h + lo : 512 * h + 512],
                            start=(c == 0),
                            stop=(c == nch - 1),
                        )

                pts = {}
                for c in range(nch):
                    pts[c] = qk_exp(c)
                    if c >= filler_delay:
                        pull(2)
                    if hooks and c in hooks:
                        hooks[c]()
                    if c >= 2:
                        pv(c - 2, pts.pop(c - 2))
                        if c >= filler_delay:
                            pull(1)
                pv(nch - 2, pts.pop(nch - 2))
                pull(2)
                pv(nch - 1, pts.pop(nch - 1))
                # psum -> SBUF copies on DVE release the opsum banks for the
                # next tile without loading the exp-critical scalar queue
                o65s = []
                for h in range(2):
                    o65 = sb.tile([65, 512], F32, tag="o65", bufs=4, name=f"o65{b}{h}{j}")
                    nc.vector.tensor_copy(o65[:], ops[h][:])
                    o65s.append(o65)
                return o65s

            def epilogue_a(b, j, o65s):
                """Immediate post-tile work (DVE + sync only, no PE/Pool):
                gather the two denominator rows, fast reciprocal, then collect
                both rows into partition 0 (partition_broadcast sources must
                start at partition 0). Returns the deferred part-B closure,
                which runs a few chunks into the NEXT tile so the Pool queue
                (broadcasts + collective dispatch) never blocks ahead of a
                mask-mul the PE is about to need."""
                sums = sb.tile([2, 512], F32, tag="sums", bufs=3, name=f"sums{b}{j}")
                for h in range(2):
                    nc.sync.dma_start(sums[h : h + 1, :], o65s[h][64:65, :])
                rec2 = sb.tile([2, 512], F32, tag="rec4", bufs=3, name=f"rec2{b}{j}")
                nc.vector.reciprocal_approx_fast(rec2[:], sums[:])
                rb1 = sb.tile([1, 2 * 512], F32, tag="rb1", bufs=3, name=f"rb1{b}{j}")
                for h in range(2):
                    nc.sync.dma_start(
                        rb1[0:1, 512 * h : 512 * h + 512], rec2[h : h + 1, :]
                    )

                def part_b(emit_cc):
                    m0 = 16 * b + 4 * j
                    o0, g = m0 % 8, m0 // 8
                    for h in range(2):
                        rec64 = sb.tile(
                            [64, 512], F32, tag="rec64", bufs=4, name=f"rec64{b}{j}{h}"
                        )
                        nc.gpsimd.partition_broadcast(
                            rec64[:], rb1[0:1, 512 * h : 512 * h + 512]
                        )
                        onr = sb.tile([64, 512], BF16, tag="onr", bufs=4, name=f"onr{b}{j}{h}")
                        nc.vector.tensor_mul(onr[:], o65s[h][0:64, :], rec64[:])
                        # all 4 chunks in one DMA: dst rows 128*(o0+i)+64h..+64
                        nc.sync.dma_start(
                            a2a_in[g][:]
                            .rearrange("(o r) t -> r o t", r=128)[
                                64 * h : 64 * h + 64, o0 : o0 + 4, :
                            ],
                            onr[:].rearrange("p (i t) -> p i t", i=4),
                        )
                    if emit_cc:
                        a2a_call(g)()

                return part_b

            def a2a_call(g):
                def f():
                    nc.gpsimd.collective_compute(
                        "AllToAll",
                        mybir.AluOpType.bypass,
                        replica_groups=[list(range(NCORES))],
                        ins=[a2a_in[g].opt()],
                        outs=[a2a_out[g].opt()],
                    )

                return f

            # ================= schedule =================
            # prolog: three tiles emitted sequentially (attention can't start
            # before (0,0) anyway; (0,2) keeps the filler source 2 tiles ahead)
            add_tile(0, 0)
            drain()
            add_tile(0, 1)
            drain()
            add_tile(0, 2)
            drain()

            add_tile(0, 3)
            pb00 = epilogue_a(0, 0, attn_core(0, 0))
            add_tile(1, 0)
            pb01 = epilogue_a(0, 1, attn_core(0, 1, hooks={3: lambda: pb00(False)}))
            add_tile(1, 1)
            # cc0: group b0/(j0,j1) complete once pb01 stages, inside attn(0,2)
            pb02 = epilogue_a(0, 2, attn_core(0, 2, hooks={5: lambda: pb01(True)}))
            add_tile(1, 2)
            add_tile(1, 3)
            pb03 = epilogue_a(0, 3, attn_core(0, 3, hooks={5: lambda: pb02(False)}))
            pb10 = epilogue_a(1, 0, attn_core(1, 0, hooks={3: lambda: pb03(True)}))
            # cc0 finished during attn(0,3): load its output during attn(1,1).
            # sync+scalar is safe -- the collective is long done, so the
            # scalar-queue wait ahead of the (1,1) exps is ~zero.
            load_at(0, [nc.sync, nc.scalar])
            add_final_fillers(0)
            pb11 = epilogue_a(
                1, 1, attn_core(1, 1, hooks={3: lambda: pb10(False)}, filler_delay=2)
            )
            add_final_fillers(1)

            def hook12():
                pb11(True)  # cc2: group b1/(j0,j1)
                load_at(1, [nc.sync])

            # final(1) pulled only from chunk 6 on: cc1 lands mid-attn(1,2),
            # so early pulls would stall the PE on the at(1) load
            pb12 = epilogue_a(
                1, 2, attn_core(1, 2, hooks={5: hook12}, filler_delay=6)
            )
            # no fillers for attn(1,3): final(2) is reserved to overlap cc3
            pb13 = epilogue_a(1, 3, attn_core(1, 3, hooks={5: lambda: pb12(False)}))
            # tail: at(2) chunks stream while the last epilogue normalizes,
            # final(2) runs while cc3 flies, then at(3) + final(3)
            load_at(2, [nc.sync, nc.scalar])
            add_final_fillers(2)
            pull(8)
            pb13(True)  # cc3
            drain()
            load_at(3, [nc.sync, nc.scalar, nc.gpsimd])
            add_final_fillers(3)
            drain()

    nc.compile()
    return nc


def _get_compiled():
    global _COMPILED
    if _COMPILED is None:
        _COMPILED = _build()
    return _COMPILED


def _prep_in_maps(embedding_word, wq, wk, wv, wo):
    bf = ml_dtypes.bfloat16
    x = np.asarray(embedding_word, np.float32).reshape(TOK, D)
    xT = np.ascontiguousarray(x.T).astype(bf)  # [D, TOK]
    # pack to [128, (b n k t)]: token block (b,n) = 4096 contiguous cols,
    # k-chunk major inside, so every DMA row segment is contiguous
    xp = np.ascontiguousarray(
        xT.reshape(KC, 128, B, NT, 512).transpose(1, 2, 3, 0, 4).reshape(128, KC * TOK)
    )

    woT = np.asarray(wo, np.float32).T  # [D, D]
    wop = np.ascontiguousarray(
        woT.reshape(KC, 128, D).transpose(1, 0, 2).reshape(128, KC * D)
    ).astype(bf)

    # within-head row permutation: 16 re rows then 16 im rows per 32-row quadrant
    perm64 = [
        (2 * (16 * q + r) if r < 16 else 2 * (16 * q + (r - 16)) + 1)
        for q in range(2)
        for r in range(32)
    ]
    perm64 = np.asarray(perm64)

    freqs = 1.0 / (10000.0 ** (np.arange(0, DH, 2, dtype=np.float64) / DH))  # [32]
    ang = np.arange(T, dtype=np.float64)[:, None] * freqs[None, :]  # [T, 32]
    cos_t, sin_t = np.cos(ang), np.sin(ang)
    rows = np.arange(128)
    wh = rows % 64
    qd = wh // 32
    r32 = wh % 32
    dmap = 16 * qd + (r32 % 16)
    sign = np.where(r32 < 16, -1.0, 1.0)
    C = np.ascontiguousarray(cos_t[:, dmap].T).astype(bf)  # [128, T]
    S = np.ascontiguousarray((sin_t[:, dmap] * sign[None, :]).T).astype(bf)

    rr = np.arange(128)[:, None]
    cc = np.arange(128)[None, :]
    mask = np.where(cc >= rr, 1.0, 0.0).astype(bf)
    ident = np.eye(128, dtype=np.float32).astype(bf)

    wqf = np.asarray(wq, np.float32)
    wkf = np.asarray(wk, np.float32)
    wvf = np.asarray(wv, np.float32)

    def pack_w(w_c):
        # w_c: [FPC, D] -> transpose -> [D, FPC] -> [128, (k c)] SBUF layout
        wT = w_c.T
        return np.ascontiguousarray(
            wT.reshape(KC, 128, FPC).transpose(1, 0, 2).reshape(128, KC * FPC)
        ).astype(bf)

    in_maps = []
    for c in range(NCORES):
        rows_c = slice(FPC * c, FPC * c + FPC)
        wq_c = wqf[rows_c].reshape(HPC, DH, D)[:, perm64, :].reshape(FPC, D)
        wk_c = wkf[rows_c].reshape(HPC, DH, D)[:, perm64, :].reshape(FPC, D)
        wv_c = wvf[rows_c]
        in_maps.append(
            {
                "xp": xp,
                "wqp": pack_w(wq_c),
                "wkp": pack_w(wk_c),
                "wvp": pack_w(wv_c),
                "wop": wop,
                "cosC": C,
                "sinS": S,
                "mask": mask,
                "ident": ident,
            }
        )
    return in_maps


def _unshard(core_outs):
    """core_outs[c] is [TPC, D] covering token chunks {c, 8+c, 16+c, 24+c}
    (row-blocks g=0..3). Interleave back to [B, T, D]."""
    a = np.stack(core_outs, axis=0)  # [8, TPC, D]
    a = a.reshape(NCORES, 4, 128, D).transpose(1, 0, 2, 3).reshape(TOK, D)
    return np.ascontiguousarray(a.reshape(B, T, D).astype(np.float32))


def kernel(embedding_word, wq, wk, wv, wo):
    nc = _get_compiled()
    in_maps = _prep_in_maps(embedding_word, wq, wk, wv, wo)
    res = bass_utils.run_bass_kernel_spmd(nc, in_maps, core_ids=list(range(NCORES)))
    return _unshard([res.results[c]["out"] for c in range(NCORES)])
